# revision 13
# baseline (speedup 1.0000x reference)
"""CTC loss (blank = last class) on 8 TRN2 NeuronCores via Bass/Tile.

Strategy (data-parallel over batch, 32 examples/core):
  Phase A (streaming, per core):
    - DMA logits tiles [128 t, 512 c] fp32
    - PE transpose -> P^T chunks in PSUM (fp32)
    - ACT exp(PSUM) -> SBUF bf16  (P^T = exp(logits)^T, unnormalized probs)
    - PE matmul with per-example one-hot matrices (+ an all-ones column)
      -> E[t, s] = exp(logits[t, y_ext[s]]) and Z[t] = sum_c exp(logits[t, c])
    - ACT copy PSUM->SBUF bf16, DMA relayout to [example-partition, t-major]
  Phase B (recursion, DVE):
    prob-domain CTC forward with periodic max-renormalization:
      p_t = e_t * (p + shift1(p) + skip_mask * shift2(p))
    log-normalizers accumulated on ACT.
  Host: tiny final gather at s=2*label_length, logaddexp, mean over batch.

The kernel returns exactly mean(nll) as float32. Host fallback (pure numpy)
is used if the device path fails or logit_length != T.
"""

import numpy as np

try:
    import ml_dtypes

    _BF16 = np.dtype(ml_dtypes.bfloat16)
except Exception:  # pragma: no cover
    _BF16 = None

# Problem constants (hardcoded per spec)
B, T, C, L = 256, 256, 512, 64
S = 2 * L + 1  # 129
BLANK = C - 1
NCORES = 8
EXN = B // NCORES  # 32 examples per core
CH = 4  # contraction chunks of 128 over C=512
W = 133  # one-hot columns per chunk: 132 states (129 + 3 pad) + 1 ones-col
EST = 136  # Ebuf stride per timestep (133 used + pad, 4B aligned)
RENORM = 4  # renormalize every 4 steps
K0 = 1e21  # renorm target: max -> K0, keeps end-states above the FTZ floor
N_RENORM = len([t for t in range(1, T) if t % RENORM == RENORM - 1 and t != T - 1])
LV_RECOMPUTE = -110.0  # ln(v/K0) below this -> host exact recompute

_PROG = None  # cached compiled program + metadata


# ---------------------------------------------------------------- host fallback
def _host_ctc(logits, labels, label_length, logit_length):
    logits = np.asarray(logits, np.float64)
    Bb, Tt, Cc = logits.shape
    Ll = labels.shape[1]
    Ss = 2 * Ll + 1
    blank = Cc - 1
    y_ext = np.full((Bb, Ss), blank, np.int64)
    y_ext[:, 1::2] = labels.astype(np.int64)
    y_m2 = np.full((Bb, Ss), blank, np.int64)
    y_m2[:, 2:] = y_ext[:, :-2]
    s_idx = np.arange(Ss)
    skip = (s_idx[None, :] >= 2) & (y_ext != blank) & (y_ext != y_m2)

    m = logits.max(-1, keepdims=True)
    lse = m + np.log(np.exp(logits - m).sum(-1, keepdims=True))
    logp = logits - lse
    NEG = -1e30
    emit0 = np.take_along_axis(logp[:, 0, :], y_ext, axis=1)
    alpha = np.where(s_idx[None, :] <= 1, emit0, NEG)
    final = np.full((Bb, Ss), NEG)
    tlast = np.asarray(logit_length, np.int64) - 1
    if np.any(tlast == 0):
        final[tlast == 0] = alpha[tlast == 0]
    for t in range(1, Tt):
        emit = np.take_along_axis(logp[:, t, :], y_ext, axis=1)
        a1 = np.concatenate([np.full((Bb, 1), NEG), alpha[:, :-1]], axis=1)
        a2 = np.concatenate([np.full((Bb, 2), NEG), alpha[:, :-2]], axis=1)
        a2 = np.where(skip, a2, NEG)
        alpha = np.logaddexp(np.logaddexp(alpha, a1), a2) + emit
        sel = tlast == t
        if np.any(sel):
            final[sel] = alpha[sel]
    b = np.arange(Bb)
    end = 2 * np.asarray(label_length, np.int64)
    nll = -np.logaddexp(final[b, end], final[b, end - 1])
    return np.float32(nll.mean())


# ---------------------------------------------------------------- device program
def _build_program():
    import concourse.bass as bass
    from concourse import bacc, mybir, masks
    import concourse.tile as tile

    f32 = mybir.dt.float32
    bf16 = mybir.dt.bfloat16
    AF = mybir.ActivationFunctionType

    nc = bacc.Bacc(
        "TRN2",
        target_bir_lowering=False,
        debug=False,
        enable_asserts=False,
        num_devices=NCORES,
    )

    logits_d = nc.dram_tensor("logits", [EXN, T, C], f32, kind="ExternalInput").ap()
    yrows_d = nc.dram_tensor("yrows", [EXN, CH * W], f32, kind="ExternalInput").ap()
    iota_d = nc.dram_tensor("iota", [128, CH * W], f32, kind="ExternalInput").ap()
    skipm_d = nc.dram_tensor("skipm", [EXN, 132], bf16, kind="ExternalInput").ap()
    out_d = nc.dram_tensor("out", [EXN, 132], f32, kind="ExternalOutput").ap()

    with tile.TileContext(nc) as tc:
        from contextlib import ExitStack

        ctx = ExitStack()
        with ctx:
            cpool = ctx.enter_context(tc.tile_pool(name="consts", bufs=1))
            ohpool = ctx.enter_context(tc.tile_pool(name="onehots", bufs=EXN))
            psumB = ctx.enter_context(tc.tile_pool(name="psB", bufs=2, space="PSUM"))
            lpool = ctx.enter_context(tc.tile_pool(name="ltiles", bufs=3))
            ptpool = ctx.enter_context(tc.tile_pool(name="pt", bufs=3))
            esbpool = ctx.enter_context(tc.tile_pool(name="esb", bufs=3))
            epool = ctx.enter_context(tc.tile_pool(name="ebuf", bufs=2))
            rpool = ctx.enter_context(tc.tile_pool(name="rec", bufs=1))
            psumT = ctx.enter_context(tc.tile_pool(name="psT", bufs=2, space="PSUM"))
            psumE = ctx.enter_context(tc.tile_pool(name="psE", bufs=2, space="PSUM"))

            # ---- constants
            iota_sb = cpool.tile([128, CH * W], f32, tag="iota")
            nc.sync.dma_start(out=iota_sb[:], in_=iota_d[:])
            # all example label-rows flattened on partition 0 (PE rhs needs base partition 0)
            yrows_sb = cpool.tile([1, EXN * CH * W], f32, tag="yrows")
            nc.sync.dma_start(
                out=yrows_sb[:], in_=yrows_d[:].rearrange("a b -> (a b)")
            )
            skipm_sb = cpool.tile([EXN, 132], bf16, tag="skipm")
            nc.sync.dma_start(out=skipm_sb[:], in_=skipm_d[:])
            ident = cpool.tile([128, 128], f32, tag="ident")
            masks.make_identity(nc, ident[:])
            ones_row = cpool.tile([1, 128], f32, tag="ones")
            nc.vector.memset(ones_row[:], 1.0)

            # ---- one-hot matrices per example: oh[c, 133*k + j] over 4 chunks
            # broadcast y-row across partitions via PE outer product, then
            # compare against the iota pattern on DVE.
            ohs = []
            for ex in range(EXN):
                oh = ohpool.tile([128, CH * W], bf16, tag="oh")
                for k in range(CH):
                    yb = psumB.tile([128, W], f32, tag="yb")
                    off = (ex * CH + k) * W
                    nc.tensor.matmul(
                        yb[:],
                        ones_row[:],
                        yrows_sb[0:1, off : off + W],
                        start=True,
                        stop=True,
                    )
                    nc.vector.tensor_tensor(
                        out=oh[:, W * k : W * (k + 1)],
                        in0=iota_sb[:, W * k : W * (k + 1)],
                        in1=yb[:],
                        op=mybir.AluOpType.is_equal,
                    )
                ohs.append(oh)

            # ---- phase A: emissions per (t-half, example)
            ebufs = []
            lzs = []
            for tb in range(2):
                ebuf = epool.tile([EXN, 128 * EST], bf16, tag="ebuf")
                for ex in range(EXN):
                    ltile = lpool.tile([128, C], f32, tag="lt")
                    nc.sync.dma_start(
                        out=ltile[:],
                        in_=logits_d[ex, tb * 128 : (tb + 1) * 128, :],
                    )
                    pT = psumT.tile([128, C], f32, tag="psT")
                    for k in range(CH):
                        nc.tensor.matmul(
                            pT[:, 128 * k : 128 * (k + 1)],
                            ltile[:, 128 * k : 128 * (k + 1)],
                            ident[:],
                            is_transpose=True,
                            start=True,
                            stop=True,
                        )
                    pt_sb = ptpool.tile([128, C], bf16, tag="pt")
                    nc.scalar.activation(pt_sb[:], pT[:], AF.Exp)
                    pE = psumE.tile([128, W], f32, tag="psE")
                    for k in range(CH):
                        nc.tensor.matmul(
                            pE[:],
                            pt_sb[:, 128 * k : 128 * (k + 1)],
                            ohs[ex][:, W * k : W * (k + 1)],
                            start=(k == 0),
                            stop=(k == CH - 1),
                        )
                    esb = esbpool.tile([128, W], bf16, tag="esb")
                    nc.scalar.copy(esb[:], pE[:])
                    # relayout: [t-part, s] -> Ebuf[ex-part, t*EST + s]
                    dst = ebuf[ex : ex + 1, :].rearrange(
                        "p (t s) -> p t s", s=EST
                    )[:, :, 0:W]
                    nc.sync.dma_start(out=dst, in_=esb[:, 0:W])
                ebufs.append(ebuf)
                # LZ for this half: ln of Z column (s-slot 132), summed over t
                zcol = ebuf[:, :].rearrange("p (t s) -> p t s", s=EST)[:, :, 132]
                lzrow = rpool.tile([EXN, 128], f32, tag="lzrow")
                nc.scalar.activation(lzrow[:], zcol, AF.Ln)
                lzh = rpool.tile([EXN, 1], f32, tag=f"lzh{tb}")
                nc.vector.reduce_sum(lzh[:], lzrow[:], axis=mybir.AxisListType.X)
                lzs.append(lzh)

            # ---- phase B: recursion
            pA = rpool.tile([EXN, 132], bf16, tag="pA")
            pB = rpool.tile([EXN, 132], bf16, tag="pB")
            xt = rpool.tile([EXN, 132], bf16, tag="xt")
            ut = rpool.tile([EXN, 132], bf16, tag="ut")
            vt = rpool.tile([EXN, 132], bf16, tag="vt")
            m8 = rpool.tile([EXN, 1], f32, tag="m8")
            rcp = rpool.tile([EXN, 1], f32, tag="rcp")
            lnm = rpool.tile([EXN, 1], f32, tag="lnm")
            accA = rpool.tile([EXN, 1], f32, tag="accA")
            accB = rpool.tile([EXN, 1], f32, tag="accB")
            nc.vector.memset(pA[:], 0.0)
            nc.vector.memset(pB[:], 0.0)
            nc.vector.memset(accA[:], 0.0)
            nc.vector.memset(accB[:], 0.0)

            # init: alpha_0(s) = e_0(s) for s in {0, 1}
            nc.vector.tensor_copy(pA[:, 2:4], ebufs[0][:, 0:2])

            pcur, pnxt = pA, pB
            acur, anxt = accA, accB
            for t in range(1, T):
                half = t // 128
                toff = (t % 128) * EST
                e_ap = ebufs[half][:, toff : toff + 129]
                nc.vector.tensor_mul(xt[:, 0:129], skipm_sb[:, 0:129], pcur[:, 0:129])
                nc.vector.tensor_add(ut[:, 0:129], pcur[:, 2:131], pcur[:, 1:130])
                nc.vector.tensor_add(vt[:, 0:129], ut[:, 0:129], xt[:, 0:129])
                nc.vector.tensor_mul(pnxt[:, 2:131], vt[:, 0:129], e_ap)
                if t % RENORM == RENORM - 1 and t != T - 1:
                    nc.vector.reduce_max(m8[:], pnxt[:, 2:131], axis=mybir.AxisListType.X)
                    nc.vector.reciprocal(rcp[:], m8[:])
                    nc.vector.tensor_scalar(
                        pnxt[:, 2:132],
                        pnxt[:, 2:132],
                        rcp[:],
                        float(K0),
                        op0=mybir.AluOpType.mult,
                        op1=mybir.AluOpType.mult,
                    )
                    # m8 can exceed ACT Ln's 2^64 range: ln(m8) = ln(m8*2^-64) + 64*ln2,
                    # the constant is added back on the host
                    nc.scalar.activation(lnm[:], m8[:], AF.Ln, scale=float(2.0**-64))
                    nc.scalar.activation(anxt[:], acur[:], AF.Identity, bias=lnm[:])
                    acur, anxt = anxt, acur
                pcur, pnxt = pnxt, pcur

            # ---- final assembly: [p_final(129) | pad | acc | LZ]
            outsb = rpool.tile([EXN, 132], f32, tag="outsb")
            nc.vector.memset(outsb[:], 0.0)
            nc.scalar.copy(outsb[:, 0:129], pcur[:, 2:131])
            nc.vector.tensor_copy(outsb[:, 130:131], acur[:])
            nc.vector.tensor_add(outsb[:, 131:132], lzs[0][:], lzs[1][:])
            nc.sync.dma_start(out=out_d[:], in_=outsb[:])

    nc.compile()
    return nc


def _get_program():
    global _PROG
    if _PROG is None:
        _PROG = _build_program()
    return _PROG


def _prep_inputs(logits, labels):
    """Build per-core in_maps."""
    logits = np.ascontiguousarray(np.asarray(logits, np.float32))
    labels = np.asarray(labels)

    y_ext = np.full((B, S), BLANK, np.int64)
    y_ext[:, 1::2] = labels.astype(np.int64)
    y_m2 = np.full((B, S), BLANK, np.int64)
    y_m2[:, 2:] = y_ext[:, :-2]
    s_idx = np.arange(S)
    skip = (s_idx[None, :] >= 2) & (y_ext != BLANK) & (y_ext != y_m2)

    skipm = np.zeros((B, 132), np.float32)
    skipm[:, 0:S] = skip.astype(np.float32)
    skipm = skipm.astype(_BF16)

    # yrows: per chunk ck, col W*ck + j: y_ext[j] - 128*ck (j<129), -1000 pads,
    # col W*ck + 132 = -1 (matches iota's -1 -> all-ones Z column)
    yrows = np.full((B, CH * W), -1000.0, np.float32)
    for ck in range(CH):
        yrows[:, W * ck : W * ck + S] = y_ext.astype(np.float32) - 128.0 * ck
        yrows[:, W * ck + 132] = -1.0

    iota = np.empty((128, CH * W), np.float32)
    p = np.arange(128, dtype=np.float32)[:, None]
    for ck in range(CH):
        iota[:, W * ck : W * ck + 132] = p
        iota[:, W * ck + 132] = -1.0

    in_maps = []
    for i in range(NCORES):
        sl = slice(i * EXN, (i + 1) * EXN)
        in_maps.append(
            {
                "logits": logits[sl],
                "yrows": yrows[sl],
                "iota": iota,
                "skipm": skipm[sl],
            }
        )
    return in_maps


def _run_device(in_maps, trace=False):
    from concourse import bass_utils

    nc = _get_program()
    res = bass_utils.run_bass_kernel_spmd(
        nc, in_maps, core_ids=list(range(NCORES)), trace=trace
    )
    return res


def _host_nll_subset(logits, labels, label_length):
    """Exact per-example nll (log-domain, float64) for a subset; full T."""
    lg = np.asarray(logits, np.float64)
    Bb = lg.shape[0]
    m = lg.max(-1, keepdims=True)
    lse = m + np.log(np.exp(lg - m).sum(-1, keepdims=True))
    logp = lg - lse
    y_ext = np.full((Bb, S), BLANK, np.int64)
    y_ext[:, 1::2] = np.asarray(labels, np.int64)
    y_m2 = np.full((Bb, S), BLANK, np.int64)
    y_m2[:, 2:] = y_ext[:, :-2]
    s_idx = np.arange(S)
    skip = (s_idx[None, :] >= 2) & (y_ext != BLANK) & (y_ext != y_m2)
    NEG = -1e30
    emit0 = np.take_along_axis(logp[:, 0, :], y_ext, axis=1)
    alpha = np.where(s_idx[None, :] <= 1, emit0, NEG)
    for t in range(1, T):
        emit = np.take_along_axis(logp[:, t, :], y_ext, axis=1)
        a1 = np.concatenate([np.full((Bb, 1), NEG), alpha[:, :-1]], axis=1)
        a2 = np.concatenate([np.full((Bb, 2), NEG), alpha[:, :-2]], axis=1)
        a2 = np.where(skip, a2, NEG)
        alpha = np.logaddexp(np.logaddexp(alpha, a1), a2) + emit
    b = np.arange(Bb)
    end = 2 * np.asarray(label_length, np.int64)
    return -np.logaddexp(alpha[b, end], alpha[b, end - 1])


def _finalize(results, logits, labels, label_length):
    label_length = np.asarray(label_length, np.int64)
    lnK0 = np.log(K0)
    nll = np.empty(B, np.float64)
    for i in range(NCORES):
        out = np.asarray(results[i]["out"], np.float64)  # [32, 132]
        pf = out[:, 0:129]
        acc = out[:, 130]
        lz = out[:, 131]
        sl = slice(i * EXN, (i + 1) * EXN)
        ll = label_length[sl]
        b = np.arange(EXN)
        v = np.maximum(pf[b, 2 * ll] + pf[b, 2 * ll - 1], 1e-300)
        # device acc accumulated ln(m8 * 2^-64); add back N_RENORM * 64 * ln2
        acc_true = acc + N_RENORM * 64.0 * np.log(2.0)
        nll[sl] = lz - (np.log(v) + acc_true - N_RENORM * lnK0)
        # examples whose end-state mass sat too close to the FTZ floor are
        # unreliable -> recompute exactly on host (rare)
        bad = (np.log(v) - lnK0) < LV_RECOMPUTE
        if np.any(bad):
            idx = np.where(bad)[0]
            nll[i * EXN + idx] = _host_nll_subset(
                logits[sl][idx], labels[sl][idx], ll[idx]
            )
    return np.float32(nll.mean())


def kernel(logits, labels, label_length, logit_length):
    logits = np.asarray(logits)
    labels = np.asarray(labels)
    label_length = np.asarray(label_length)
    logit_length = np.asarray(logit_length)

    if (
        logits.shape != (B, T, C)
        or labels.shape != (B, L)
        or not np.all(np.asarray(logit_length) == T)
        or _BF16 is None
    ):
        return _host_ctc(logits, labels, label_length, logit_length)

    try:
        in_maps = _prep_inputs(logits, labels)
        res = _run_device(in_maps, trace=False)
        return _finalize(res.results, logits, labels, label_length)
    except Exception:
        import traceback

        traceback.print_exc()
        return _host_ctc(logits, labels, label_length, logit_length)
